# revision 3
# baseline (speedup 1.0000x reference)
"""Distance-discriminator kernel for 8 Trainium2 cores (bf16 pipeline).

Math (reference): for x [N, D],
    S[d] = sum_j x[j,d];  Q[d] = sum_j x[j,d]^2
    sq[i,d] = Q[d] - 2 x[i,d] S[d] + N x[i,d]^2      (= sum_j (x[j,d]-x[i,d])^2)
    out = log(sqrt(sq) + eps) @ W.T + b

Device formulation: z''[i,d] = x^2 - (2S/N) x, so sq = N z'' + Q and
    logd2 = ln(sq) = Ln(N*EMC0 * z'' + EMC0*Q) + C0
with the C0 centering and eps folded into the host-side weights/bias
(dist ~ sqrt(2N) >> eps).

Columns d are sharded across the 8 cores (512 each): S, Q are local and no
mid-kernel communication is needed; each core emits a [128, 2048]-packed
bf16 partial of out.T and the host sums/unpacks. Inputs are cast to bf16
on the host, halving HBM traffic (the 2e-2 tolerance leaves ample room).

Engine split per 128-partition chunk [128, N]:
  DVE: tensor_scalar (junk out, accum -> -2S/N)   -- the only stat needed
       scalar_tensor_tensor z'' = (x + a) * x, accum -> Q - 2S^2/N
       tiny [128,1] ops to form the Ln bias EMC0*Q
  ACT: Ln only (its 1x, dtype-independent rate makes it the scarce engine)
  PE : bf16 GEMM, PSUM banks packed 2 j-blocks deep (partitions 0:64/64:128)
  DVE/ACT: PSUM evacuation + bias, bf16 out.
"""

import numpy as np
import ml_dtypes

import concourse.bacc as bacc
import concourse.bass as bass
import concourse.tile as tile
from concourse import mybir
from concourse.tile import add_dep_helper
from concourse.bass_utils import run_bass_kernel_spmd

N = 4096          # rows
D = 4096          # feature columns
OUT = 64
NCORES = 8
DC = D // NCORES  # 512 columns per core
KCH = DC // 128   # 4 partition-chunks per core
NBLK = 8          # 512-wide j-blocks per core
C0 = 8.9          # ln(sq) centering constant; absorbed via host bias
EMC0 = float(np.exp(-C0))

F32 = mybir.dt.float32
BF16 = mybir.dt.bfloat16
BF = ml_dtypes.bfloat16
_cache: dict = {}


def _build():
    nc = bacc.Bacc(
        "TRN2",
        target_bir_lowering=False,
        debug=False,
        num_devices=NCORES,
    )
    xT = nc.dram_tensor("xT", [DC, N], BF16, kind="ExternalInput").ap()
    wT = nc.dram_tensor("wT", [128, KCH * OUT], BF16, kind="ExternalInput").ap()
    bb = nc.dram_tensor("bb", [128, 1], F32, kind="ExternalInput").ap()
    out = nc.dram_tensor("out", [128, NBLK // 2 * 512], BF16, kind="ExternalOutput").ap()

    MUL = mybir.AluOpType.mult
    ADD = mybir.AluOpType.add
    with tile.TileContext(nc) as tc:
        with (
            tc.tile_pool(name="wp", bufs=1) as wp,
            tc.tile_pool(name="xp", bufs=KCH) as xp,
            tc.tile_pool(name="zp", bufs=KCH) as zp,
            tc.tile_pool(name="lp", bufs=KCH) as lp,
            tc.tile_pool(name="st", bufs=KCH) as st,
            tc.tile_pool(name="pp", bufs=4, space="PSUM") as pp,
        ):
            # preload the Ln table set while ACT is idle during the DMA phase
            dumm = wp.tile([128, 1], F32, name="dumm", tag="dumm")
            nc.vector.memset(dumm[:], 1.0)
            dumm2 = wp.tile([128, 1], F32, name="dumm2", tag="dumm2")
            pre_ln = nc.scalar.activation(
                dumm2[:], dumm[:], mybir.ActivationFunctionType.Ln,
                bias=dumm[:], scale=1.0,
            )

            # x DMA: 2 pieces per chunk, alternating the two HWDGE queues so
            # both run at line rate; chunk k completes at ~ (k+1)/4 of the
            # DMA window, so stats/square/Ln pipeline chunk-by-chunk.
            xs = []
            for k in range(KCH):
                x_k = xp.tile([128, N], BF16, name=f"x_{k}", tag="x")
                npieces = 4 if k == 0 else 2
                w_piece = N // npieces
                for s in range(npieces):
                    eng = nc.scalar if s % 2 else nc.sync
                    eng.dma_start(
                        x_k[:, s * w_piece : (s + 1) * w_piece],
                        xT[k * 128 : (k + 1) * 128, s * w_piece : (s + 1) * w_piece],
                    )
                xs.append(x_k)

            w_all = wp.tile([128, KCH * OUT], BF16, name="w_all", tag="w_all")
            nc.sync.dma_start(w_all[:], wT)
            bias_b = wp.tile([128, 1], F32, name="bias_b", tag="bias_b")
            nc.sync.dma_start(bias_b[:], bb)

            junk = wp.tile([128, N // 2], BF16, name="junk", tag="junk")

            zs, biasQs = [], []
            for k in range(KCH):
                x_k = xs[k]
                # pass 1: accum gives -2S/N; the data output is discarded
                accs = []
                np1 = 2
                wp1 = N // np1
                for s in range(np1):
                    a_s = st.tile([128, 1], F32, name=f"a_{k}_{s}", tag="acc")
                    nc.vector.tensor_scalar(
                        junk[:, :wp1], x_k[:, s * wp1 : (s + 1) * wp1],
                        -2.0 / N, 0.0, op0=MUL, op1=ADD, accum_out=a_s[:],
                    )
                    accs.append(a_s)
                a_k = st.tile([128, 1], F32, name=f"a_{k}", tag="a")
                nc.vector.tensor_tensor(a_k[:], accs[0][:], accs[1][:], op=ADD)
                # pass 2: z'' = (x + a) * x ; accum -> sum z'' = Q - 2S^2/N
                z_k = zp.tile([128, N], BF16, name=f"z_{k}", tag="z")
                zsums = []
                np2 = 2 if k == 0 else 1
                wp2 = N // np2
                for s in range(np2):
                    zs_s = st.tile([128, 1], F32, name=f"zs_{k}_{s}", tag="zsum")
                    nc.vector.scalar_tensor_tensor(
                        z_k[:, s * wp2 : (s + 1) * wp2],
                        x_k[:, s * wp2 : (s + 1) * wp2],
                        a_k[:],
                        x_k[:, s * wp2 : (s + 1) * wp2],
                        op0=ADD, op1=MUL, accum_out=zs_s[:],
                    )
                    zsums.append(zs_s)
                if np2 == 2:
                    zsum_k = st.tile([128, 1], F32, name=f"zsum_{k}", tag="zsc")
                    nc.vector.tensor_tensor(
                        zsum_k[:], zsums[0][:], zsums[1][:], op=ADD
                    )
                else:
                    zsum_k = zsums[0]
                # bias = EMC0*Q ; Q = zsum + 2S^2/N = zsum + (N/2)*a^2
                t1_k = st.tile([128, 1], F32, name=f"t1_{k}", tag="t1")
                nc.vector.tensor_tensor(t1_k[:], a_k[:], a_k[:], op=MUL)
                q_k = st.tile([128, 1], F32, name=f"q_{k}", tag="q")
                nc.vector.scalar_tensor_tensor(
                    q_k[:], t1_k[:], float(N) / 2.0, zsum_k[:], op0=MUL, op1=ADD,
                )
                biasQ_k = st.tile([128, 1], F32, name=f"biasQ_{k}", tag="bQ")
                nc.vector.tensor_scalar(
                    biasQ_k[:], q_k[:], EMC0, None, op0=MUL,
                )
                zs.append(z_k)
                biasQs.append(biasQ_k)

            # Ln + GEMM + evac. PSUM bank b holds j-blocks (2b, 2b+1) packed on
            # partitions 0:64 / 64:128; host unpacks.
            psums = [
                pp.tile([128, 512], F32, name=f"ps_{b}", tag="ps")
                for b in range(4)
            ]
            out_sb = wp.tile([128, 4 * 512], BF16, name="out_sb", tag="out_sb")
            LNSCALE = float(N) * EMC0
            for k in range(KCH):
                # finer Ln pieces on the last chunk shorten the tail drain
                bounds = (
                    [0, 1024, 4096] if k == 0
                    else ([0, 2048, 3072, 4096] if k == KCH - 1 else [0, 2048, 4096])
                )
                l_k = lp.tile([128, N], BF16, name=f"l_{k}", tag="l")
                for p in range(len(bounds) - 1):
                    lo, hi = bounds[p], bounds[p + 1]
                    act = nc.scalar.activation(
                        l_k[:, lo:hi], zs[k][:, lo:hi],
                        mybir.ActivationFunctionType.Ln,
                        bias=biasQs[k][:], scale=LNSCALE,
                    )
                    if k == 0 and p == 0:
                        add_dep_helper(
                            act.ins, pre_ln.ins, sync=False,
                            reason="table preload first",
                        )
                    for j in range(lo // 512, hi // 512):
                        b, h = j // 2, j % 2
                        nc.tensor.matmul(
                            psums[b][h * 64 : (h + 1) * 64, :],
                            lhsT=w_all[:, k * OUT : (k + 1) * OUT],
                            rhs=l_k[:, j * 512 : (j + 1) * 512],
                            start=(k == 0),
                            stop=(k == KCH - 1),
                        )

            for b in range(4):
                nc.vector.tensor_scalar(
                    out_sb[:, b * 512 : (b + 1) * 512], psums[b][:],
                    bias_b[:], None, op0=ADD,
                )
                nc.sync.dma_start(
                    out[:, b * 512 : (b + 1) * 512],
                    out_sb[:, b * 512 : (b + 1) * 512],
                )

    nc.compile()
    return nc


def _prep_inputs(data, W, b):
    data = np.asarray(data, dtype=np.float32)
    W = np.asarray(W, dtype=np.float32)
    b = np.asarray(b, dtype=np.float32)
    dataT = np.ascontiguousarray(data.T)               # [D, N]
    W2T = W.T * 0.5                                    # [D, OUT]
    in_maps = []
    for c in range(NCORES):
        xT_c = dataT[c * DC : (c + 1) * DC].astype(BF)             # [DC, N]
        w_c = W2T[c * DC : (c + 1) * DC, :].astype(BF)             # [DC, OUT]
        # [128, KCH*OUT]: chunk k's weight block side by side
        wT_c = np.ascontiguousarray(
            w_c.reshape(KCH, 128, OUT).transpose(1, 0, 2).reshape(128, KCH * OUT)
        )
        # bias per core: b/8 plus the centering correction C0*sum_d w2[d,o],
        # stacked twice for the partition-packed PSUM layout
        b8_c = (b / NCORES + C0 * w_c.astype(np.float32).sum(axis=0)).astype(np.float32)
        bb_c = np.ascontiguousarray(
            np.concatenate([b8_c, b8_c]).reshape(128, 1)
        )
        in_maps.append({"xT": xT_c, "wT": wT_c, "bb": bb_c})
    return in_maps


def _run(inputs, trace=False, **kwargs):
    if "nc" not in _cache:
        _cache["nc"] = _build()
    nc = _cache["nc"]
    in_maps = _prep_inputs(inputs["data"], inputs["W"], inputs["b"])
    res = run_bass_kernel_spmd(
        nc, in_maps, core_ids=list(range(NCORES)), trace=trace, **kwargs
    )
    # out[c] is [128, 2048] bf16: bank b cols [512b,512b+512), partitions
    # 0:64 -> j-block 2b, 64:128 -> j-block 2b+1
    acc = np.zeros((128, 2048), dtype=np.float32)
    for c in range(NCORES):
        acc += np.asarray(res.results[c]["out"]).astype(np.float32)
    outT = np.empty((OUT, N), dtype=np.float32)
    for b in range(4):
        outT[:, (2 * b) * 512 : (2 * b + 1) * 512] = acc[0:64, b * 512 : (b + 1) * 512]
        outT[:, (2 * b + 1) * 512 : (2 * b + 2) * 512] = acc[64:128, b * 512 : (b + 1) * 512]
    return np.ascontiguousarray(outT.T), res


def kernel(data, W, b):
    out, _ = _run({"data": data, "W": W, "b": b})
    return out


# revision 5
# speedup vs baseline: 1.1573x; 1.1573x over previous
"""Distance-discriminator kernel for 8 Trainium2 cores (bf16 pipeline).

Math (reference): for x [N, D],
    S[d] = sum_j x[j,d];  Q[d] = sum_j x[j,d]^2
    sq[i,d] = Q[d] - 2 x[i,d] S[d] + N x[i,d]^2      (= sum_j (x[j,d]-x[i,d])^2)
    out = log(sqrt(sq) + eps) @ W.T + b

Device formulation: complete the square,
    u = (sqrt(N) x - S/sqrt(N))^2,  sq = u + C,  C = Q - S^2/N = (sum_j u)/N
    logd2 = ln(sq) = Ln(EMC0*u + EMC0*C) + C0
with the C0 centering and eps folded into host-side weights/bias.

Columns d are sharded across the 8 cores (512 each): S, Q stay local, no
mid-kernel communication. Inputs are cast to bf16 on the host, halving HBM
traffic (tolerance 2e-2 leaves ample room; measured ~3e-3).

Engine split (rates measured on HW: DVE ts 4x bf16, tt 2x, everything with
an accumulator 1x; ACT 1x dtype-independent):
  S per chunk: DVE pairwise tt-fold 4096->512 at 2x, then one 1x reduce.
  chunks 0,1:  ACT Square(scale=sqrt(N), bias=-S/sqrt(N)) with accum -> N*C.
  chunks 2,3:  DVE v = ts(x*sqrt(N) + bA) at 4x, then tensor_tensor_reduce
               u = (v*v)*EMC0 with accum at 1x (cheaper than tt + fold).
  ACT: Ln over every chunk (the scarce resource), bias = EMC0*C.
  PE:  bf16 GEMM, PSUM banks packed 2 j-blocks deep (partitions 0:64/64:128).
"""

import contextlib

import numpy as np
import ml_dtypes

import concourse.bacc as bacc
import concourse.bass as bass
import concourse.tile as tile
from concourse import mybir
from concourse.tile import add_dep_helper
from concourse.bass_utils import run_bass_kernel_spmd

N = 4096          # rows
D = 4096          # feature columns
OUT = 64
NCORES = 8
DC = D // NCORES  # 512 columns per core
KCH = DC // 128   # 4 partition-chunks per core
SQRT_N = float(np.sqrt(N))
C0 = 8.9          # ln(sq) centering constant; absorbed via host bias
EMC0 = float(np.exp(-C0))
NZ = 2            # chunks squared on ACT (with free C via accum)

F32 = mybir.dt.float32
BF16 = mybir.dt.bfloat16
BF = ml_dtypes.bfloat16
_cache: dict = {}


def _build():
    nc = bacc.Bacc(
        "TRN2",
        target_bir_lowering=False,
        debug=False,
        num_devices=NCORES,
    )
    xT = nc.dram_tensor("xT", [DC, N], BF16, kind="ExternalInput").ap()
    wT = nc.dram_tensor("wT", [128, KCH * OUT], BF16, kind="ExternalInput").ap()
    bb = nc.dram_tensor("bb", [128, 1], F32, kind="ExternalInput").ap()
    out = nc.dram_tensor("out", [128, 4 * 512], BF16, kind="ExternalOutput").ap()

    MUL = mybir.AluOpType.mult
    ADD = mybir.AluOpType.add
    with tile.TileContext(nc) as tc:
        with (
            tc.tile_pool(name="wp", bufs=1) as wp,
            tc.tile_pool(name="xp", bufs=KCH) as xp,
            tc.tile_pool(name="up", bufs=KCH) as up,
            tc.tile_pool(name="lp", bufs=KCH) as lp,
            tc.tile_pool(name="st", bufs=KCH) as st,
            tc.tile_pool(name="pp", bufs=4, space="PSUM") as pp,
        ):
            # preload the Ln/Square table sets while ACT idles in the DMA phase
            dumm = wp.tile([128, 1], F32, name="dumm", tag="dumm")
            nc.vector.memset(dumm[:], 1.0)
            dumm2 = wp.tile([128, 1], F32, name="dumm2", tag="dumm2")
            pre_sq = nc.scalar.activation(
                dumm2[:], dumm[:], mybir.ActivationFunctionType.Square,
            )
            pre_ln = nc.scalar.activation(
                dumm2[:], dumm[:], mybir.ActivationFunctionType.Ln,
                bias=dumm[:], scale=1.0,
            )

            # x DMA: alternate the two HWDGE queues; chunk k lands at
            # ~(k+1)/4 of the DMA window.
            xs = []
            for k in range(KCH):
                x_k = xp.tile([128, N], BF16, name=f"x_{k}", tag="x")
                npieces = 4 if k == 0 else 2
                w_piece = N // npieces
                for s in range(npieces):
                    eng = nc.scalar if s % 2 else nc.sync
                    eng.dma_start(
                        x_k[:, s * w_piece : (s + 1) * w_piece],
                        xT[k * 128 : (k + 1) * 128, s * w_piece : (s + 1) * w_piece],
                    )
                xs.append(x_k)

            w_all = wp.tile([128, KCH * OUT], BF16, name="w_all", tag="w_all")
            nc.sync.dma_start(w_all[:], wT)
            bias_b = wp.tile([128, 1], F32, name="bias_b", tag="bias_b")
            nc.sync.dma_start(bias_b[:], bb)

            us, biasLns = [], []
            for k in range(KCH):
                x_k = xs[k]
                prio = tc.high_priority() if k < 2 else contextlib.nullcontext()
                with prio:
                    # S: pairwise fold at 2x down to 512, one 1x reduce
                    f1 = st.tile([128, 2048], BF16, name=f"f1_{k}", tag="f1")
                    if k == 0:
                        # c0 DMA'd in 4 pieces: fold (p0,p2) and (p1,p3) as
                        # they land to cut the S latency
                        nc.vector.tensor_tensor(
                            f1[:, :1024], x_k[:, 0:1024], x_k[:, 2048:3072], op=ADD
                        )
                        nc.vector.tensor_tensor(
                            f1[:, 1024:], x_k[:, 1024:2048], x_k[:, 3072:4096], op=ADD
                        )
                    else:
                        nc.vector.tensor_tensor(
                            f1[:], x_k[:, :2048], x_k[:, 2048:], op=ADD
                        )
                    f2 = st.tile([128, 1024], BF16, name=f"f2_{k}", tag="f2")
                    nc.vector.tensor_tensor(
                        f2[:], f1[:, :1024], f1[:, 1024:], op=ADD
                    )
                    f3 = st.tile([128, 512], BF16, name=f"f3_{k}", tag="f3")
                    nc.vector.tensor_tensor(
                        f3[:], f2[:, :512], f2[:, 512:], op=ADD
                    )
                    s_k = st.tile([128, 1], F32, name=f"s_{k}", tag="s")
                    nc.vector.tensor_reduce(
                        s_k[:], f3[:], axis=mybir.AxisListType.X, op=ADD
                    )
                    bA_k = st.tile([128, 1], F32, name=f"bA_{k}", tag="bA")
                    nc.vector.tensor_scalar(
                        bA_k[:], s_k[:], -1.0 / SQRT_N, None, op0=MUL
                    )

                u_k = up.tile([128, N], BF16, name=f"u_{k}", tag="u")
                acc_k = st.tile([128, 1], F32, name=f"acc_{k}", tag="acc")
                if k < NZ:
                    # ACT square: u = (sqrt(N) x + bA)^2, accum -> N*C
                    act = nc.scalar.activation(
                        u_k[:], x_k[:],
                        mybir.ActivationFunctionType.Square,
                        bias=bA_k[:], scale=SQRT_N, accum_out=acc_k[:],
                    )
                    if k == 0:
                        add_dep_helper(act.ins, pre_sq.ins, sync=False,
                                       reason="table preload first")
                        add_dep_helper(act.ins, pre_ln.ins, sync=False,
                                       reason="table preload first")
                    lnscale = EMC0
                    accscale = EMC0 / N
                else:
                    # DVE square: v at 4x, u = v*v at 2x; C = sum(u) via a
                    # second pairwise fold (tensor_tensor_reduce hangs the HW)
                    v_k = st.tile([128, N], BF16, name=f"v_{k}", tag="v")
                    nc.vector.tensor_scalar(
                        v_k[:], x_k[:], SQRT_N, bA_k[:], op0=MUL, op1=ADD
                    )
                    nc.vector.tensor_tensor(u_k[:], v_k[:], v_k[:], op=MUL)
                    g1 = st.tile([128, 2048], BF16, name=f"g1_{k}", tag="g1")
                    nc.vector.tensor_tensor(
                        g1[:], u_k[:, :2048], u_k[:, 2048:], op=ADD
                    )
                    g2 = st.tile([128, 1024], BF16, name=f"g2_{k}", tag="g2")
                    nc.vector.tensor_tensor(
                        g2[:], g1[:, :1024], g1[:, 1024:], op=ADD
                    )
                    g3 = st.tile([128, 512], BF16, name=f"g3_{k}", tag="g3")
                    nc.vector.tensor_tensor(
                        g3[:], g2[:, :512], g2[:, 512:], op=ADD
                    )
                    nc.vector.tensor_reduce(
                        acc_k[:], g3[:], axis=mybir.AxisListType.X, op=ADD
                    )
                    lnscale = EMC0
                    accscale = EMC0 / N
                with tc.high_priority():
                    biasLn_k = st.tile([128, 1], F32, name=f"biasLn_{k}", tag="bL")
                    nc.vector.tensor_scalar(
                        biasLn_k[:], acc_k[:], accscale, None, op0=MUL
                    )
                us.append((u_k, lnscale))
                biasLns.append(biasLn_k)

            # Ln + GEMM + evac. PSUM bank b holds j-blocks (2b, 2b+1) on
            # partitions 0:64 / 64:128; host unpacks.
            psums = [
                pp.tile([128, 512], F32, name=f"ps_{b}", tag="ps")
                for b in range(4)
            ]
            out_sb = wp.tile([128, 4 * 512], BF16, name="out_sb", tag="out_sb")
            for k in range(KCH):
                bounds = [0, 2048, 3072, 4096] if k == KCH - 1 else [0, 2048, 4096]
                u_k, lnscale = us[k]
                l_k = lp.tile([128, N], BF16, name=f"l_{k}", tag="l")
                for p in range(len(bounds) - 1):
                    lo, hi = bounds[p], bounds[p + 1]
                    nc.scalar.activation(
                        l_k[:, lo:hi], u_k[:, lo:hi],
                        mybir.ActivationFunctionType.Ln,
                        bias=biasLns[k][:], scale=lnscale,
                    )
                    for j in range(lo // 512, hi // 512):
                        b, h = j // 2, j % 2
                        nc.tensor.matmul(
                            psums[b][h * 64 : (h + 1) * 64, :],
                            lhsT=w_all[:, k * OUT : (k + 1) * OUT],
                            rhs=l_k[:, j * 512 : (j + 1) * 512],
                            start=(k == 0),
                            stop=(k == KCH - 1),
                        )

            for b in range(4):
                with tc.high_priority():
                    nc.vector.tensor_scalar(
                        out_sb[:, b * 512 : (b + 1) * 512], psums[b][:],
                        bias_b[:], None, op0=ADD,
                    )
                nc.sync.dma_start(
                    out[:, b * 512 : (b + 1) * 512],
                    out_sb[:, b * 512 : (b + 1) * 512],
                )

    nc.compile()
    return nc


def _prep_inputs(data, W, b):
    data = np.asarray(data, dtype=np.float32)
    W = np.asarray(W, dtype=np.float32)
    b = np.asarray(b, dtype=np.float32)
    dataT = np.ascontiguousarray(data.T)               # [D, N]
    W2T = W.T * 0.5                                    # [D, OUT]
    in_maps = []
    for c in range(NCORES):
        xT_c = dataT[c * DC : (c + 1) * DC].astype(BF)             # [DC, N]
        w_c = W2T[c * DC : (c + 1) * DC, :].astype(BF)             # [DC, OUT]
        wT_c = np.ascontiguousarray(
            w_c.reshape(KCH, 128, OUT).transpose(1, 0, 2).reshape(128, KCH * OUT)
        )
        # bias per core: b/8 plus the centering correction C0*sum_d w2[d,o],
        # stacked twice for the partition-packed PSUM layout
        b8_c = (b / NCORES + C0 * w_c.astype(np.float32).sum(axis=0)).astype(np.float32)
        bb_c = np.ascontiguousarray(
            np.concatenate([b8_c, b8_c]).reshape(128, 1)
        )
        in_maps.append({"xT": xT_c, "wT": wT_c, "bb": bb_c})
    return in_maps


def _run(inputs, trace=False, **kwargs):
    if "nc" not in _cache:
        _cache["nc"] = _build()
    nc = _cache["nc"]
    in_maps = _prep_inputs(inputs["data"], inputs["W"], inputs["b"])
    res = run_bass_kernel_spmd(
        nc, in_maps, core_ids=list(range(NCORES)), trace=trace, **kwargs
    )
    acc = np.zeros((128, 2048), dtype=np.float32)
    for c in range(NCORES):
        acc += np.asarray(res.results[c]["out"]).astype(np.float32)
    outT = np.empty((OUT, N), dtype=np.float32)
    for b in range(4):
        outT[:, (2 * b) * 512 : (2 * b + 1) * 512] = acc[0:64, b * 512 : (b + 1) * 512]
        outT[:, (2 * b + 1) * 512 : (2 * b + 2) * 512] = acc[64:128, b * 512 : (b + 1) * 512]
    return np.ascontiguousarray(outT.T), res


def kernel(data, W, b):
    out, _ = _run({"data": data, "W": W, "b": b})
    return out
